# revision 6
# baseline (speedup 1.0000x reference)
"""AttentionalGNN Trainium2 kernel — 8-core SPMD.

Sharding: core c = (b, q) with b = c // 4 (batch), q = c % 4 (node quarter,
256 of 1024 nodes). Every core runs an identical program; per-core behavior
differs only through input data (its batch's desc tensors and its node
slice). Per layer:
  - k/v^T convs computed on the full node axis (replicated within a batch
    group), q/MLP/attention computed for the local node quarter,
  - BatchNorm statistics AllReduce'd across all 8 cores (4 KB),
  - layer outputs AllGather'd within each batch group of 4 (2 MB)
    to rebuild the full-node stream slabs for the next layer's k/v.
Matmuls run as float32r (full-rate fp32). Softmax uses no max-subtraction
(|scores| <= ~64 for this model, exp stays in fp32 range); the per-node
softmax denominator comes from a ones-column folded into v^T, and the
division is folded into the PSUM->SBUF evacuation of the message matmul.
"""

import numpy as np

import concourse.bass as bass
import concourse.tile as tile
from concourse import bacc, mybir
from concourse.bass_utils import run_bass_kernel_spmd

L, D, H, B, N = 18, 256, 4, 2, 1024
HD = D // H           # 64
NL = N // 4           # 256 local nodes per core
EPS = 1e-5
F32 = mybir.dt.float32
F32R = mybir.dt.float32r
AF = mybir.ActivationFunctionType
OP = mybir.AluOpType

# head-contiguous channel permutation: perm[h*64+hd] = hd*4+h
PERM = np.array([hd * H + h for h in range(H) for hd in range(HD)], np.int64)

_CACHE = {}


def _r(ap):
    return ap.bitcast(F32R)


def _build_program():
    nc = bacc.Bacc("TRN2", target_bir_lowering=False, debug=False, num_devices=8)

    dram = {}
    def din(name, shape):
        dram[name] = nc.dram_tensor(name, shape, F32, kind="ExternalInput")
    din("wqt", [L, 2, 128, 256]); din("wkt", [L, 2, 128, 256])
    din("wvt", [L, 2, 128, 256]); din("wmt", [L, 2, 128, 256])
    din("w1t", [L, 4, 128, 512]); din("w2t", [L, 4, 128, 256])
    din("bq", [L, 2, 128]); din("bm", [L, 2, 128])
    din("b1", [L, 4, 128]); din("b2", [L, 2, 128])
    din("g1d", [L, 8, 128]); din("be1d", [L, 8, 128])
    din("d0f", [256, N]); din("d1f", [256, N])
    din("x0", [256, NL]); din("x1", [256, NL])
    out_d = nc.dram_tensor("out", [L, 2, 2, 128, NL], F32, kind="ExternalOutput")

    RG_ALL = [list(range(8))]
    RG_B = [[0, 1, 2, 3], [4, 5, 6, 7]]

    with tile.TileContext(nc) as tc:
        from contextlib import ExitStack
        _es = ExitStack()
        wp = _es.enter_context(tc.tile_pool(name="wp", bufs=2))
        a2 = _es.enter_context(tc.tile_pool(name="a2", bufs=2))
        a1 = _es.enter_context(tc.tile_pool(name="a1", bufs=1))
        ep = _es.enter_context(tc.tile_pool(name="ep", bufs=10))
        p512 = _es.enter_context(tc.tile_pool(name="p512", bufs=2, space="PSUM"))
        p256 = _es.enter_context(tc.tile_pool(name="p256", bufs=4, space="PSUM"))
        pmsg = _es.enter_context(tc.tile_pool(name="pmsg", bufs=2, space="PSUM"))
        dp = _es.enter_context(tc.tile_pool(name="dp", bufs=2, space="DRAM"))

        # ---- persistent initial state ----
        # stream slabs (full node axis, own batch): sA = stream0, sB = stream1
        slabs = [[None, None], [None, None]]
        for s, nm in ((0, "d0f"), (1, "d1f")):
            for c in range(2):
                t = a2.tile([128, N], F32, tag=f"s{s}{c}", name=f"s{s}{c}_init")
                nc.sync.dma_start(out=_r(t[:]), in_=_r(dram[nm].ap()[c * 128:(c + 1) * 128, :]))
                slabs[s][c] = t
        # local x slices per stream
        x = [[None, None], [None, None]]
        desc_loc = [[None, None], [None, None]]
        for s, nm in ((0, "x0"), (1, "x1")):
            for c in range(2):
                t = a1.tile([128, NL], F32, tag=f"dl{s}{c}", name=f"dl{s}{c}")
                nc.sync.dma_start(out=_r(t[:]), in_=_r(dram[nm].ap()[c * 128:(c + 1) * 128, :]))
                desc_loc[s][c] = t
                x[s][c] = t

        for i in range(L):
            # ---- weight loads ----
            def wload(nm, kc, cols):
                t = wp.tile([128, kc, cols], F32, tag=nm, name=f"{nm}_{i}")
                nc.sync.dma_start(out=_r(t[:]), in_=_r(dram[nm].ap()[i].rearrange("k p o -> p k o")))
                return t
            wq_t = wload("wqt", 2, 256); wk_t = wload("wkt", 2, 256)
            wv_t = wload("wvt", 2, 256); wm_t = wload("wmt", 2, 256)
            w1_t = wload("w1t", 4, 512); w2_t = wload("w2t", 4, 256)
            def bload(nm, kc):
                t = wp.tile([128, kc], F32, tag=nm, name=f"{nm}_{i}")
                nc.sync.dma_start(out=t[:], in_=dram[nm].ap()[i].rearrange("k p -> p k"))
                return t
            bq_t = bload("bq", 2); bm_t = bload("bm", 2)
            b1_t = bload("b1", 4); b2_t = bload("b2", 2)
            g1_t = bload("g1d", 8); be1_t = bload("be1d", 8)

            # source slab per stream (static per layer)
            if i == 0:
                srcs = (slabs[0], slabs[1])
            elif i == 1:
                srcs = (slabs[1], slabs[0])
            elif i % 2 == 0:
                srcs = (slabs[0], slabs[1])
            else:
                srcs = (slabs[1], slabs[0])

            qt = [[None, None], [None, None]]
            kt = [[None, None], [None, None]]
            vt = [[None] * 8, [None] * 8]
            for u in (0, 1):
                src = srcs[u]
                # q = WqT.T @ x + bq   [256, NL]
                for mo in range(2):
                    ps = p256.tile([128, NL], F32, tag="p256", name=f"qp{i}{u}{mo}")
                    for k in range(2):
                        nc.tensor.matmul(ps[:], _r(wq_t[:, k, mo * 128:(mo + 1) * 128]),
                                         _r(x[u][k][:]), start=(k == 0), stop=(k == 1))
                    t = a2.tile([128, NL], F32, tag=f"q{u}{mo}", name=f"q{i}{u}{mo}")
                    nc.vector.tensor_scalar(_r(t[:]), ps[:], bq_t[:, mo:mo + 1], None, OP.add)
                    qt[u][mo] = t
                # k = WkT.T @ src   [256, N] (no bias)
                for mo in range(2):
                    t = a1.tile([128, N], F32, tag=f"k{u}{mo}", name=f"k{i}{u}{mo}")
                    for nn in range(2):
                        ps = p512.tile([128, 512], F32, tag="p512", name=f"kp{i}{u}{mo}{nn}")
                        for k in range(2):
                            nc.tensor.matmul(ps[:], _r(wk_t[:, k, mo * 128:(mo + 1) * 128]),
                                             _r(src[k][:, nn * 512:(nn + 1) * 512]),
                                             start=(k == 0), stop=(k == 1))
                        nc.vector.tensor_copy(_r(t[:, nn * 512:(nn + 1) * 512]), ps[:])
                    kt[u][mo] = t
                # vT folds: vT[f] = src[:, f*128:(f+1)*128].T @ WvT  -> [128, 4*65]
                for f in range(8):
                    ps = p256.tile([128, 256], F32, tag="p256", name=f"vp{i}{u}{f}")
                    for k in range(2):
                        nc.tensor.matmul(ps[:], _r(src[k][:, f * 128:(f + 1) * 128]),
                                         _r(wv_t[:, k, :]), start=(k == 0), stop=(k == 1))
                    t = a1.tile([128, 260], F32, tag=f"v{u}{f}", name=f"v{i}{u}{f}")
                    tv = t[:].rearrange("p (h c) -> p h c", h=4)
                    nc.vector.memset(tv[:, :, 64:65], 1.0)
                    nc.vector.tensor_copy(_r(tv[:, :, 0:64]),
                                          ps[:].rearrange("p (h c) -> p h c", c=64))
                    vt[u][f] = t

            # ---- attention: 8 units ----
            msgt = [[None, None], [None, None]]
            for u in (0, 1):
                for c in range(2):
                    msgt[u][c] = a2.tile([128, NL], F32, tag=f"m{u}{c}", name=f"m{i}{u}{c}")
            for u in (0, 1):
                for h in range(H):
                    kt_t = kt[u][h // 2]
                    q_t = qt[u][h // 2]
                    r0 = (h % 2) * 64
                    mg = pmsg.tile([65, NL], F32, tag="pmsg", name=f"mg{i}{u}{h}")
                    for f in range(8):
                        sc = p256.tile([128, NL], F32, tag="p256", name=f"sc{i}{u}{h}{f}")
                        nc.tensor.matmul(sc[:], _r(kt_t[r0:r0 + 64, f * 128:(f + 1) * 128]),
                                         _r(q_t[r0:r0 + 64, :]), start=True, stop=True)
                        ex = ep.tile([128, NL], F32, tag="ep", name=f"ex{i}{u}{h}{f}")
                        nc.scalar.activation(_r(ex[:]), sc[:], AF.Exp)
                        nc.tensor.matmul(mg[:], _r(vt[u][f][:, h * 65:(h + 1) * 65]),
                                         _r(ex[:]), start=(f == 0), stop=(f == 7))
                    rec = a2.tile([1, NL], F32, tag="rec", name=f"rec{i}{u}{h}")
                    nc.vector.reciprocal(rec[:], mg[64:65, :])
                    rbc = a2.tile([64, NL], F32, tag="rbc", name=f"rbc{i}{u}{h}")
                    nc.gpsimd.partition_broadcast(rbc[:], rec[:])
                    nc.vector.tensor_tensor(_r(msgt[u][h // 2][r0:r0 + 64, :]),
                                            mg[0:64, :], rbc[:], OP.mult)

            # ---- MLP ----
            statsloc = a2.tile([128, 2, 8], F32, tag="stl", name=f"stl{i}")
            ht = [[None] * 4, [None] * 4]
            sq_scratch = a2.tile([128, NL], F32, tag="sqs", name=f"sqs{i}")
            for u in (0, 1):
                # msgc = WmT.T @ msg + bm_eff
                msgc = [None, None]
                for mo in range(2):
                    ps = p256.tile([128, NL], F32, tag="p256", name=f"cp{i}{u}{mo}")
                    for k in range(2):
                        nc.tensor.matmul(ps[:], _r(wm_t[:, k, mo * 128:(mo + 1) * 128]),
                                         _r(msgt[u][k][:]), start=(k == 0), stop=(k == 1))
                    t = a2.tile([128, NL], F32, tag=f"mc{u}{mo}", name=f"mc{i}{u}{mo}")
                    nc.vector.tensor_scalar(_r(t[:]), ps[:], bm_t[:, mo:mo + 1], None, OP.add)
                    msgc[mo] = t
                ych = [x[u][0], x[u][1], msgc[0], msgc[1]]
                # conv1 + bias (+ stats via accum)
                for mo in range(4):
                    ps = p256.tile([128, NL], F32, tag="p256", name=f"h1p{i}{u}{mo}")
                    for k in range(4):
                        nc.tensor.matmul(ps[:], _r(w1_t[:, k, mo * 128:(mo + 1) * 128]),
                                         _r(ych[k][:]), start=(k == 0), stop=(k == 3))
                    t = a1.tile([128, NL], F32, tag=f"h{u}{mo}", name=f"h{i}{u}{mo}")
                    j = u * 4 + mo
                    nc.scalar.activation(t[:], ps[:], AF.Identity,
                                         bias=b1_t[:, mo:mo + 1],
                                         accum_out=statsloc[:, 0, j:j + 1])
                    nc.scalar.activation(sq_scratch[:], t[:], AF.Square,
                                         accum_out=statsloc[:, 1, j:j + 1])
                    ht[u][mo] = t

            # ---- BN stats AllReduce (all 8 cores) ----
            bn_in = dp.tile([128, 2, 8], F32, tag="bni", name=f"bni{i}")
            bn_out = dp.tile([128, 2, 8], F32, tag="bno", name=f"bno{i}")
            nc.sync.dma_start(out=bn_in[:], in_=statsloc[:])
            nc.gpsimd.collective_compute("AllReduce", OP.add, replica_groups=RG_ALL,
                                         ins=[bn_in[:].opt()], outs=[bn_out[:].opt()])
            stg = a2.tile([128, 2, 8], F32, tag="stg", name=f"stg{i}")
            nc.sync.dma_start(out=stg[:], in_=bn_out[:])

            # scale/shift: mean = sum/2048; var = sumsq/2048 - mean^2
            mean_t = a2.tile([128, 8], F32, tag="mean", name=f"mean{i}")
            var_t = a2.tile([128, 8], F32, tag="var", name=f"var{i}")
            sc_t = a2.tile([128, 8], F32, tag="scl", name=f"scl{i}")
            sh_t = a2.tile([128, 8], F32, tag="shf", name=f"shf{i}")
            nc.vector.tensor_scalar(mean_t[:], stg[:, 0, :], 1.0 / 2048.0, None, OP.mult)
            nc.vector.tensor_scalar(var_t[:], stg[:, 1, :], 1.0 / 2048.0, None, OP.mult)
            nc.vector.tensor_tensor(sc_t[:], mean_t[:], mean_t[:], OP.mult)
            nc.vector.tensor_tensor(var_t[:], var_t[:], sc_t[:], OP.subtract)
            nc.vector.tensor_scalar(var_t[:], var_t[:], EPS, None, OP.add)
            nc.scalar.activation(var_t[:], var_t[:], AF.Ln)
            nc.scalar.activation(var_t[:], var_t[:], AF.Exp, scale=-0.5)  # rsqrt
            nc.vector.tensor_tensor(sc_t[:], var_t[:], g1_t[:], OP.mult)
            nc.vector.tensor_tensor(sh_t[:], mean_t[:], sc_t[:], OP.mult)
            nc.vector.tensor_tensor(sh_t[:], be1_t[:], sh_t[:], OP.subtract)

            # ---- BN apply + relu + conv2 + residual ----
            ag_in = dp.tile([2, 2, 128, NL], F32, tag="agi", name=f"agi{i}")
            ag_out = dp.tile([4, 2, 2, 128, NL], F32, tag="ago", name=f"ago{i}")
            for u in (0, 1):
                hn = [None] * 4
                for mo in range(4):
                    j = u * 4 + mo
                    t = a1.tile([128, NL], F32, tag=f"hn{u}{mo}", name=f"hn{i}{u}{mo}")
                    nc.scalar.activation(_r(t[:]), ht[u][mo][:], AF.Relu,
                                         bias=sh_t[:, j:j + 1], scale=sc_t[:, j:j + 1])
                    hn[mo] = t
                for mo in range(2):
                    ps = p256.tile([128, NL], F32, tag="p256", name=f"o2p{i}{u}{mo}")
                    for k in range(4):
                        nc.tensor.matmul(ps[:], _r(w2_t[:, k, mo * 128:(mo + 1) * 128]),
                                         _r(hn[k][:]), start=(k == 0), stop=(k == 3))
                    xn = a2.tile([128, NL], F32, tag=f"x{u}{mo}", name=f"x{i}{u}{mo}")
                    nc.vector.tensor_scalar(_r(xn[:]), ps[:], b2_t[:, mo:mo + 1], None, OP.add)
                    resid = desc_loc[u][mo] if i <= 1 else x[u][mo]
                    nc.vector.tensor_tensor(_r(xn[:]), xn[:], resid[:], OP.add)
                    nc.sync.dma_start(out=out_d.ap()[i][u][mo], in_=xn[:])
                    nc.sync.dma_start(out=ag_in[u, mo], in_=xn[:])
                    x[u][mo] = xn

            # ---- layer-output AllGather within batch group ----
            if i < L - 1:
                nc.gpsimd.collective_compute("AllGather", OP.bypass, replica_groups=RG_B,
                                             ins=[ag_in[:].opt()], outs=[ag_out[:].opt()])
                for s in range(2):
                    for c in range(2):
                        t = a2.tile([128, N], F32, tag=f"s{s}{c}", name=f"s{s}{c}_{i}")
                        for qq in range(4):
                            nc.sync.dma_start(out=_r(t[:, qq * NL:(qq + 1) * NL]),
                                              in_=_r(ag_out[qq, s, c]))
                        slabs[s][c] = t

        _es.close()

    nc.finalize()
    return nc


def _host_prep(inputs):
    f = np.float32
    Wq, bq = np.asarray(inputs["Wq"], f), np.asarray(inputs["bq"], f)
    Wk = np.asarray(inputs["Wk"], f)
    Wv, bv = np.asarray(inputs["Wv"], f), np.asarray(inputs["bv"], f)
    Wm, bm = np.asarray(inputs["Wm"], f), np.asarray(inputs["bm"], f)
    W1, b1 = np.asarray(inputs["W1"], f), np.asarray(inputs["b1"], f)
    g1, be1 = np.asarray(inputs["g1"], f), np.asarray(inputs["be1"], f)
    W2, b2 = np.asarray(inputs["W2"], f), np.asarray(inputs["b2"], f)
    d0, d1 = np.asarray(inputs["desc0"], f), np.asarray(inputs["desc1"], f)

    SCALE = 1.0 / np.sqrt(HD).astype(f)
    wqt = np.ascontiguousarray(
        (Wq[:, PERM, :] * SCALE).transpose(0, 2, 1).reshape(L, 2, 128, 256))
    wkt = np.ascontiguousarray(Wk[:, PERM, :].transpose(0, 2, 1).reshape(L, 2, 128, 256))
    wvt = np.ascontiguousarray(Wv[:, PERM, :].transpose(0, 2, 1).reshape(L, 2, 128, 256))
    wmt = np.ascontiguousarray(Wm[:, :, PERM].transpose(0, 2, 1).reshape(L, 2, 128, 256))
    w1t = np.ascontiguousarray(W1.transpose(0, 2, 1).reshape(L, 4, 128, 512))
    w2t = np.ascontiguousarray(W2.transpose(0, 2, 1).reshape(L, 4, 128, 256))
    bq_a = np.ascontiguousarray((bq[:, PERM] * SCALE).reshape(L, 2, 128))
    bm_eff = np.einsum("loi,li->lo", Wm, bv) + bm
    bm_a = np.ascontiguousarray(bm_eff.astype(f).reshape(L, 2, 128))
    b1_a = np.ascontiguousarray(b1.reshape(L, 4, 128))
    b2_a = np.ascontiguousarray(b2.reshape(L, 2, 128))
    g1d = np.ascontiguousarray(np.tile(g1.reshape(L, 1, 4, 128), (1, 2, 1, 1)).reshape(L, 8, 128))
    be1d = np.ascontiguousarray(np.tile(be1.reshape(L, 1, 4, 128), (1, 2, 1, 1)).reshape(L, 8, 128))

    shared = dict(wqt=wqt, wkt=wkt, wvt=wvt, wmt=wmt, w1t=w1t, w2t=w2t,
                  bq=bq_a, bm=bm_a, b1=b1_a, b2=b2_a, g1d=g1d, be1d=be1d)
    in_maps = []
    for c in range(8):
        b, q = c // 4, c % 4
        m = dict(shared)
        m["d0f"] = np.ascontiguousarray(d0[b])
        m["d1f"] = np.ascontiguousarray(d1[b])
        m["x0"] = np.ascontiguousarray(d0[b][:, q * NL:(q + 1) * NL])
        m["x1"] = np.ascontiguousarray(d1[b][:, q * NL:(q + 1) * NL])
        in_maps.append(m)
    return in_maps, d0, d1


def kernel(**inputs):
    if "nc" not in _CACHE:
        _CACHE["nc"] = _build_program()
    nc = _CACHE["nc"]
    in_maps, d0, d1 = _host_prep(inputs)
    res = run_bass_kernel_spmd(nc, in_maps, list(range(8)))

    outs = [np.zeros((B, D, N), np.float32) for _ in range(2 * L + 2)]
    outs[2] = d0.copy(); outs[3] = d1.copy()
    for c in range(8):
        b, q = c // 4, c % 4
        O = res.results[c]["out"]  # [L, 2, 2, 128, NL]
        for i in range(L):
            for u in range(2):
                j = u if i == 0 else (4 + u if i == 1 else 2 * i + 2 + u)
                outs[j][b, :, q * NL:(q + 1) * NL] = O[i, u].reshape(D, NL)
    return tuple(outs)


# revision 7
# speedup vs baseline: 83906903.3194x; 83906903.3194x over previous
"""AttentionalGNN Trainium2 kernel — 8-core SPMD.

Sharding: core c = (b, q) with b = c // 4 (batch), q = c % 4 (node quarter,
256 of 1024 nodes). Every core runs an identical program; per-core behavior
differs only through input data (its batch's desc tensors and its node
slice). Per layer:
  - k/v^T convs computed on the full node axis (replicated within a batch
    group), q/MLP/attention computed for the local node quarter,
  - BatchNorm statistics AllReduce'd across all 8 cores (4 KB),
  - layer outputs AllGather'd within each batch group of 4 (2 MB)
    to rebuild the full-node stream slabs for the next layer's k/v.
Matmuls run as float32r (full-rate fp32). Softmax uses no max-subtraction
(|scores| <= ~64 for this model, exp stays in fp32 range); the per-node
softmax denominator comes from a ones-column folded into v^T, and the
division is folded into the PSUM->SBUF evacuation of the message matmul.
"""

import numpy as np

import concourse.bass as bass
import concourse.tile as tile
from concourse import bacc, mybir
from concourse.bass_utils import run_bass_kernel_spmd

L, D, H, B, N = 18, 256, 4, 2, 1024
HD = D // H           # 64
NL = N // 4           # 256 local nodes per core
EPS = 1e-5
F32 = mybir.dt.float32
F32R = mybir.dt.float32r
AF = mybir.ActivationFunctionType
OP = mybir.AluOpType

# head-contiguous channel permutation: perm[h*64+hd] = hd*4+h
PERM = np.array([hd * H + h for h in range(H) for hd in range(HD)], np.int64)

_CACHE = {}


def _r(ap):
    return ap.bitcast(F32R)


def _build_program(n_layers=L, use_coll=True, num_devices=8):
    nc = bacc.Bacc("TRN2", target_bir_lowering=False, debug=False, num_devices=num_devices)

    dram = {}
    def din(name, shape):
        dram[name] = nc.dram_tensor(name, shape, F32, kind="ExternalInput")
    din("wqt", [L, 2, 128, 256]); din("wkt", [L, 2, 128, 256])
    din("wvt", [L, 2, 128, 256]); din("wmt", [L, 2, 128, 256])
    din("w1t", [L, 4, 128, 512]); din("w2t", [L, 4, 128, 256])
    din("bq", [L, 2, 128]); din("bm", [L, 2, 128])
    din("b1", [L, 4, 128]); din("b2", [L, 2, 128])
    din("g1d", [L, 8, 128]); din("be1d", [L, 8, 128])
    din("d0f", [256, N]); din("d1f", [256, N])
    din("x0", [256, NL]); din("x1", [256, NL])
    out_d = nc.dram_tensor("out", [L, 2, 2, 128, NL], F32, kind="ExternalOutput")

    RG_ALL = [list(range(8))]
    RG_B = [[0, 1, 2, 3], [4, 5, 6, 7]]

    with tile.TileContext(nc) as tc:
        from contextlib import ExitStack
        _es = ExitStack()
        wp = _es.enter_context(tc.tile_pool(name="wp", bufs=2))
        a2 = _es.enter_context(tc.tile_pool(name="a2", bufs=2))
        a1 = _es.enter_context(tc.tile_pool(name="a1", bufs=1))
        ep = _es.enter_context(tc.tile_pool(name="ep", bufs=10))
        p512 = _es.enter_context(tc.tile_pool(name="p512", bufs=2, space="PSUM"))
        p256 = _es.enter_context(tc.tile_pool(name="p256", bufs=4, space="PSUM"))
        pmsg = _es.enter_context(tc.tile_pool(name="pmsg", bufs=2, space="PSUM"))
        dp = _es.enter_context(tc.tile_pool(name="dp", bufs=2, space="DRAM"))

        # ---- persistent initial state ----
        # stream slabs (full node axis, own batch): sA = stream0, sB = stream1
        slabs = [[None, None], [None, None]]
        for s, nm in ((0, "d0f"), (1, "d1f")):
            for c in range(2):
                t = a2.tile([128, N], F32, tag=f"s{s}{c}", name=f"s{s}{c}_init")
                nc.sync.dma_start(out=_r(t[:]), in_=_r(dram[nm].ap()[c * 128:(c + 1) * 128, :]))
                slabs[s][c] = t
        # local x slices per stream
        x = [[None, None], [None, None]]
        desc_loc = [[None, None], [None, None]]
        for s, nm in ((0, "x0"), (1, "x1")):
            for c in range(2):
                t = a1.tile([128, NL], F32, tag=f"dl{s}{c}", name=f"dl{s}{c}")
                nc.sync.dma_start(out=_r(t[:]), in_=_r(dram[nm].ap()[c * 128:(c + 1) * 128, :]))
                desc_loc[s][c] = t
                x[s][c] = t

        for i in range(n_layers):
            # ---- weight loads ----
            def wload(nm, kc, cols):
                t = wp.tile([128, kc, cols], F32, tag=nm, name=f"{nm}_{i}")
                nc.sync.dma_start(out=_r(t[:]), in_=_r(dram[nm].ap()[i].rearrange("k p o -> p k o")))
                return t
            wq_t = wload("wqt", 2, 256); wk_t = wload("wkt", 2, 256)
            wv_t = wload("wvt", 2, 256); wm_t = wload("wmt", 2, 256)
            w1_t = wload("w1t", 4, 512); w2_t = wload("w2t", 4, 256)
            def bload(nm, kc):
                t = wp.tile([128, kc], F32, tag=nm, name=f"{nm}_{i}")
                nc.sync.dma_start(out=t[:], in_=dram[nm].ap()[i].rearrange("k p -> p k"))
                return t
            bq_t = bload("bq", 2); bm_t = bload("bm", 2)
            b1_t = bload("b1", 4); b2_t = bload("b2", 2)
            g1_t = bload("g1d", 8); be1_t = bload("be1d", 8)

            # source slab per stream (static per layer)
            if i == 0:
                srcs = (slabs[0], slabs[1])
            elif i == 1:
                srcs = (slabs[1], slabs[0])
            elif i % 2 == 0:
                srcs = (slabs[0], slabs[1])
            else:
                srcs = (slabs[1], slabs[0])

            qt = [[None, None], [None, None]]
            kt = [[None, None], [None, None]]
            vt = [[None] * 8, [None] * 8]
            for u in (0, 1):
                src = srcs[u]
                # q = WqT.T @ x + bq   [256, NL]
                for mo in range(2):
                    ps = p256.tile([128, NL], F32, tag="p256", name=f"qp{i}{u}{mo}")
                    for k in range(2):
                        nc.tensor.matmul(ps[:], _r(wq_t[:, k, mo * 128:(mo + 1) * 128]),
                                         _r(x[u][k][:]), start=(k == 0), stop=(k == 1))
                    t = a2.tile([128, NL], F32, tag=f"q{u}{mo}", name=f"q{i}{u}{mo}")
                    nc.vector.tensor_scalar(_r(t[:]), ps[:], bq_t[:, mo:mo + 1], None, OP.add)
                    qt[u][mo] = t
                # k = WkT.T @ src   [256, N] (no bias)
                for mo in range(2):
                    t = a1.tile([128, N], F32, tag=f"k{u}{mo}", name=f"k{i}{u}{mo}")
                    for nn in range(2):
                        ps = p512.tile([128, 512], F32, tag="p512", name=f"kp{i}{u}{mo}{nn}")
                        for k in range(2):
                            nc.tensor.matmul(ps[:], _r(wk_t[:, k, mo * 128:(mo + 1) * 128]),
                                             _r(src[k][:, nn * 512:(nn + 1) * 512]),
                                             start=(k == 0), stop=(k == 1))
                        nc.vector.tensor_copy(_r(t[:, nn * 512:(nn + 1) * 512]), ps[:])
                    kt[u][mo] = t
                # vT folds: vT[f] = src[:, f*128:(f+1)*128].T @ WvT  -> [128, 4*65]
                for f in range(8):
                    ps = p256.tile([128, 256], F32, tag="p256", name=f"vp{i}{u}{f}")
                    for k in range(2):
                        nc.tensor.matmul(ps[:], _r(src[k][:, f * 128:(f + 1) * 128]),
                                         _r(wv_t[:, k, :]), start=(k == 0), stop=(k == 1))
                    t = a1.tile([128, 260], F32, tag=f"v{u}{f}", name=f"v{i}{u}{f}")
                    tv = t[:].rearrange("p (h c) -> p h c", h=4)
                    nc.vector.memset(tv[:, :, 64:65], 1.0)
                    nc.vector.tensor_copy(_r(tv[:, :, 0:64]),
                                          ps[:].rearrange("p (h c) -> p h c", c=64))
                    vt[u][f] = t

            # ---- attention: 8 units ----
            msgt = [[None, None], [None, None]]
            for u in (0, 1):
                for c in range(2):
                    msgt[u][c] = a2.tile([128, NL], F32, tag=f"m{u}{c}", name=f"m{i}{u}{c}")
            for u in (0, 1):
                for h in range(H):
                    kt_t = kt[u][h // 2]
                    q_t = qt[u][h // 2]
                    r0 = (h % 2) * 64
                    mg = pmsg.tile([65, NL], F32, tag="pmsg", name=f"mg{i}{u}{h}")
                    for f in range(8):
                        sc = p256.tile([128, NL], F32, tag="p256", name=f"sc{i}{u}{h}{f}")
                        nc.tensor.matmul(sc[:], _r(kt_t[r0:r0 + 64, f * 128:(f + 1) * 128]),
                                         _r(q_t[r0:r0 + 64, :]), start=True, stop=True)
                        ex = ep.tile([128, NL], F32, tag="ep", name=f"ex{i}{u}{h}{f}")
                        nc.scalar.activation(_r(ex[:]), sc[:], AF.Exp)
                        nc.tensor.matmul(mg[:], _r(vt[u][f][:, h * 65:(h + 1) * 65]),
                                         _r(ex[:]), start=(f == 0), stop=(f == 7))
                    rec = a2.tile([1, NL], F32, tag="rec", name=f"rec{i}{u}{h}")
                    nc.vector.reciprocal(rec[:], mg[64:65, :])
                    rbc = a2.tile([64, NL], F32, tag="rbc", name=f"rbc{i}{u}{h}")
                    nc.gpsimd.partition_broadcast(rbc[:], rec[:])
                    nc.vector.tensor_tensor(_r(msgt[u][h // 2][r0:r0 + 64, :]),
                                            mg[0:64, :], rbc[:], OP.mult)

            # ---- MLP ----
            statsloc = a2.tile([128, 2, 8], F32, tag="stl", name=f"stl{i}")
            ht = [[None] * 4, [None] * 4]
            sq_scratch = a2.tile([128, NL], F32, tag="sqs", name=f"sqs{i}")
            for u in (0, 1):
                # msgc = WmT.T @ msg + bm_eff
                msgc = [None, None]
                for mo in range(2):
                    ps = p256.tile([128, NL], F32, tag="p256", name=f"cp{i}{u}{mo}")
                    for k in range(2):
                        nc.tensor.matmul(ps[:], _r(wm_t[:, k, mo * 128:(mo + 1) * 128]),
                                         _r(msgt[u][k][:]), start=(k == 0), stop=(k == 1))
                    t = a2.tile([128, NL], F32, tag=f"mc{u}{mo}", name=f"mc{i}{u}{mo}")
                    nc.vector.tensor_scalar(_r(t[:]), ps[:], bm_t[:, mo:mo + 1], None, OP.add)
                    msgc[mo] = t
                ych = [x[u][0], x[u][1], msgc[0], msgc[1]]
                # conv1 + bias (+ stats via accum)
                for mo in range(4):
                    ps = p256.tile([128, NL], F32, tag="p256", name=f"h1p{i}{u}{mo}")
                    for k in range(4):
                        nc.tensor.matmul(ps[:], _r(w1_t[:, k, mo * 128:(mo + 1) * 128]),
                                         _r(ych[k][:]), start=(k == 0), stop=(k == 3))
                    t = a1.tile([128, NL], F32, tag=f"h{u}{mo}", name=f"h{i}{u}{mo}")
                    j = u * 4 + mo
                    nc.scalar.activation(t[:], ps[:], AF.Identity,
                                         bias=b1_t[:, mo:mo + 1],
                                         accum_out=statsloc[:, 0, j:j + 1])
                    nc.scalar.activation(sq_scratch[:], t[:], AF.Square,
                                         accum_out=statsloc[:, 1, j:j + 1])
                    ht[u][mo] = t

            # ---- BN stats AllReduce (all 8 cores) ----
            bn_in = dp.tile([128, 2, 8], F32, tag="bni", name=f"bni{i}")
            bn_out = dp.tile([128, 2, 8], F32, tag="bno", name=f"bno{i}")
            nc.sync.dma_start(out=bn_in[:], in_=statsloc[:])
            if use_coll:
                nc.gpsimd.collective_compute("AllReduce", OP.add, replica_groups=RG_ALL,
                                             ins=[bn_in[:].opt()], outs=[bn_out[:].opt()])
            else:
                nc.sync.dma_start(out=bn_out[:], in_=bn_in[:])
            stg = a2.tile([128, 2, 8], F32, tag="stg", name=f"stg{i}")
            nc.sync.dma_start(out=stg[:], in_=bn_out[:])

            # scale/shift: mean = sum/2048; var = sumsq/2048 - mean^2
            mean_t = a2.tile([128, 8], F32, tag="mean", name=f"mean{i}")
            var_t = a2.tile([128, 8], F32, tag="var", name=f"var{i}")
            sc_t = a2.tile([128, 8], F32, tag="scl", name=f"scl{i}")
            sh_t = a2.tile([128, 8], F32, tag="shf", name=f"shf{i}")
            nc.vector.tensor_scalar(mean_t[:], stg[:, 0, :], 1.0 / 2048.0, None, OP.mult)
            nc.vector.tensor_scalar(var_t[:], stg[:, 1, :], 1.0 / 2048.0, None, OP.mult)
            nc.vector.tensor_tensor(sc_t[:], mean_t[:], mean_t[:], OP.mult)
            nc.vector.tensor_tensor(var_t[:], var_t[:], sc_t[:], OP.subtract)
            nc.vector.tensor_scalar(var_t[:], var_t[:], EPS, None, OP.add)
            nc.scalar.activation(var_t[:], var_t[:], AF.Ln)
            nc.scalar.activation(var_t[:], var_t[:], AF.Exp, scale=-0.5)  # rsqrt
            nc.vector.tensor_tensor(sc_t[:], var_t[:], g1_t[:], OP.mult)
            nc.vector.tensor_tensor(sh_t[:], mean_t[:], sc_t[:], OP.mult)
            nc.vector.tensor_tensor(sh_t[:], be1_t[:], sh_t[:], OP.subtract)

            # ---- BN apply + relu + conv2 + residual ----
            ag_in = dp.tile([2, 2, 128, NL], F32, tag="agi", name=f"agi{i}")
            ag_out = dp.tile([4, 2, 2, 128, NL], F32, tag="ago", name=f"ago{i}")
            for u in (0, 1):
                hn = [None] * 4
                for mo in range(4):
                    j = u * 4 + mo
                    t = a1.tile([128, NL], F32, tag=f"hn{u}{mo}", name=f"hn{i}{u}{mo}")
                    nc.scalar.activation(_r(t[:]), ht[u][mo][:], AF.Relu,
                                         bias=sh_t[:, j:j + 1], scale=sc_t[:, j:j + 1])
                    hn[mo] = t
                for mo in range(2):
                    ps = p256.tile([128, NL], F32, tag="p256", name=f"o2p{i}{u}{mo}")
                    for k in range(4):
                        nc.tensor.matmul(ps[:], _r(w2_t[:, k, mo * 128:(mo + 1) * 128]),
                                         _r(hn[k][:]), start=(k == 0), stop=(k == 3))
                    xn = a2.tile([128, NL], F32, tag=f"x{u}{mo}", name=f"x{i}{u}{mo}")
                    nc.vector.tensor_scalar(_r(xn[:]), ps[:], b2_t[:, mo:mo + 1], None, OP.add)
                    resid = desc_loc[u][mo] if i <= 1 else x[u][mo]
                    nc.vector.tensor_tensor(_r(xn[:]), xn[:], resid[:], OP.add)
                    nc.sync.dma_start(out=out_d.ap()[i][u][mo], in_=xn[:])
                    nc.sync.dma_start(out=ag_in[u, mo], in_=xn[:])
                    x[u][mo] = xn

            # ---- layer-output AllGather within batch group ----
            if i < n_layers - 1:
                if use_coll:
                    nc.gpsimd.collective_compute("AllGather", OP.bypass, replica_groups=RG_B,
                                                 ins=[ag_in[:].opt()], outs=[ag_out[:].opt()])
                else:
                    for qq in range(4):
                        nc.sync.dma_start(out=ag_out[qq], in_=ag_in[:])
                for s in range(2):
                    for c in range(2):
                        t = a2.tile([128, N], F32, tag=f"s{s}{c}", name=f"s{s}{c}_{i}")
                        for qq in range(4):
                            nc.sync.dma_start(out=_r(t[:, qq * NL:(qq + 1) * NL]),
                                              in_=_r(ag_out[qq, s, c]))
                        slabs[s][c] = t

        _es.close()

    nc.finalize()
    return nc


def _host_prep(inputs):
    f = np.float32
    Wq, bq = np.asarray(inputs["Wq"], f), np.asarray(inputs["bq"], f)
    Wk = np.asarray(inputs["Wk"], f)
    Wv, bv = np.asarray(inputs["Wv"], f), np.asarray(inputs["bv"], f)
    Wm, bm = np.asarray(inputs["Wm"], f), np.asarray(inputs["bm"], f)
    W1, b1 = np.asarray(inputs["W1"], f), np.asarray(inputs["b1"], f)
    g1, be1 = np.asarray(inputs["g1"], f), np.asarray(inputs["be1"], f)
    W2, b2 = np.asarray(inputs["W2"], f), np.asarray(inputs["b2"], f)
    d0, d1 = np.asarray(inputs["desc0"], f), np.asarray(inputs["desc1"], f)

    SCALE = 1.0 / np.sqrt(HD).astype(f)
    wqt = np.ascontiguousarray(
        (Wq[:, PERM, :] * SCALE).transpose(0, 2, 1).reshape(L, 2, 128, 256))
    wkt = np.ascontiguousarray(Wk[:, PERM, :].transpose(0, 2, 1).reshape(L, 2, 128, 256))
    wvt = np.ascontiguousarray(Wv[:, PERM, :].transpose(0, 2, 1).reshape(L, 2, 128, 256))
    wmt = np.ascontiguousarray(Wm[:, :, PERM].transpose(0, 2, 1).reshape(L, 2, 128, 256))
    w1t = np.ascontiguousarray(W1.transpose(0, 2, 1).reshape(L, 4, 128, 512))
    w2t = np.ascontiguousarray(W2.transpose(0, 2, 1).reshape(L, 4, 128, 256))
    bq_a = np.ascontiguousarray((bq[:, PERM] * SCALE).reshape(L, 2, 128))
    bm_eff = np.einsum("loi,li->lo", Wm, bv) + bm
    bm_a = np.ascontiguousarray(bm_eff.astype(f).reshape(L, 2, 128))
    b1_a = np.ascontiguousarray(b1.reshape(L, 4, 128))
    b2_a = np.ascontiguousarray(b2.reshape(L, 2, 128))
    g1d = np.ascontiguousarray(np.tile(g1.reshape(L, 1, 4, 128), (1, 2, 1, 1)).reshape(L, 8, 128))
    be1d = np.ascontiguousarray(np.tile(be1.reshape(L, 1, 4, 128), (1, 2, 1, 1)).reshape(L, 8, 128))

    shared = dict(wqt=wqt, wkt=wkt, wvt=wvt, wmt=wmt, w1t=w1t, w2t=w2t,
                  bq=bq_a, bm=bm_a, b1=b1_a, b2=b2_a, g1d=g1d, be1d=be1d)
    in_maps = []
    for c in range(8):
        b, q = c // 4, c % 4
        m = dict(shared)
        m["d0f"] = np.ascontiguousarray(d0[b])
        m["d1f"] = np.ascontiguousarray(d1[b])
        m["x0"] = np.ascontiguousarray(d0[b][:, q * NL:(q + 1) * NL])
        m["x1"] = np.ascontiguousarray(d1[b][:, q * NL:(q + 1) * NL])
        in_maps.append(m)
    return in_maps, d0, d1


def kernel(**inputs):
    if "nc" not in _CACHE:
        _CACHE["nc"] = _build_program()
    nc = _CACHE["nc"]
    in_maps, d0, d1 = _host_prep(inputs)
    res = run_bass_kernel_spmd(nc, in_maps, list(range(8)))

    outs = [np.zeros((B, D, N), np.float32) for _ in range(2 * L + 2)]
    outs[2] = d0.copy(); outs[3] = d1.copy()
    for c in range(8):
        b, q = c // 4, c % 4
        O = res.results[c]["out"]  # [L, 2, 2, 128, NL]
        for i in range(n_layers):
            for u in range(2):
                j = u if i == 0 else (4 + u if i == 1 else 2 * i + 2 + u)
                outs[j][b, :, q * NL:(q + 1) * NL] = O[i, u].reshape(D, NL)
    return tuple(outs)
